# revision 12
# baseline (speedup 1.0000x reference)
"""Trainium2 Bass kernel for nn_CfCDoseController.

Model: 3-layer CfC (closed-form continuous-time) RNN, AutoNCP(16,1) wiring:
  layers (fan_in, hidden): (4,9) -> (9,6) -> (6,1), T=512 steps, B=4096.
  cell: a = tanh(W1m@xx+b1); b = tanh(W2m@xx+b2); t = sigmoid(Wt@xx+bt)
        h' = a + t*(b-a)
  sigma = sigmoid(out)*sigma_scale + SIGMA_MIN;  hx = concat(final states)

Kernel strategy (pure data parallel over batch, 8 cores x 512 samples):
  * Layer-pipelined recurrence: tick s computes L0@time s, L1@s-1, L2@s-2.
    All three layers' cells then depend only on the previous tick's outputs,
    so they fuse into ONE matmul per tick.
  * sigmoid(z) = 0.5*(1 + tanh(z/2)) -> the time-gate shares the tanh op.
  * With y = tanh(z/2):  h' = 0.5*(v - u) where u = (y-1)*a, v = (y+1)*b.
    u,v are produced by a single fused DVE scalar_tensor_tensor op, and the
    0.5*(v-u) linear map is folded into the next matmul's weights.
  * Per core: 2 independent batch streams (latency hiding) x 2 blocks of 128
    samples packed on partitions. PSUM rows = [ff1(32), ff2(32), z(32), z(32)]
    (z duplicated so one STT covers both u and v), free dim = 128 samples.
  * Per stream-tick: 1 matmul + 1 tanh (ACT) + 1 STT (DVE). Everything else
    (x loads, output extraction, final sigmoid) is off the critical path.
"""

import numpy as np

try:
    import concourse.bass as bass
except Exception:  # pragma: no cover - path fallback
    import sys

    for _p in ("/opt/trn_rl_repo", "/root/.axon_site/_ro/trn_rl_repo"):
        if _p not in sys.path:
            sys.path.insert(0, _p)
    import concourse.bass as bass

import concourse.tile as tile
from concourse import bacc, bass_utils, mybir

SIGMA_MAX = 0.15
SIGMA_MIN = 0.001
LAYERS = [(4, 9), (9, 6), (6, 1)]
B, T, IN = 4096, 512, 4
NCORES = 8
BCORE = B // NCORES  # 512
S = 2  # independent streams per core
G = 2  # batch blocks per stream
F = BCORE // (S * G)  # 128 free dim (samples per block)
TICKS = 520  # 65 groups of 8 >= T + 2 pipeline tail
NG = TICKS // 8  # x/output DMA groups per stream
NRING = 6  # ring buffers per stream (8 slots each)
NCHUNK = 5  # ceil((TICKS+1)/128) slot chunks in the output stage
FP = mybir.dt.float32
FR = mybir.dt.float32r  # fp32 bits, single-pass PE matmul
BF = mybir.dt.bfloat16

_compiled = None


# ----------------------------------------------------------------------------
# host-side weight folding
# ----------------------------------------------------------------------------

def _make_rxb(Ws, bs):
    """Fold the 3 layers' (already masked) weights into the combined 16-dim
    state-space maps: pre = R @ state + X @ x + b, state=[h0(9),h1(6),h2(1)]."""
    R = np.zeros((16, 16), np.float64)
    X = np.zeros((16, 4), np.float64)
    b = np.zeros((16,), np.float64)
    W0, W1, W2 = Ws
    X[0:9, :] = W0[:, 0:4]
    R[0:9, 0:9] = W0[:, 4:13]
    R[9:15, 0:9] = W1[:, 0:9]
    R[9:15, 9:15] = W1[:, 9:15]
    R[15, 9:15] = W2[0, 0:6]
    R[15, 15] = W2[0, 6]
    b[0:9], b[9:15], b[15] = bs[0], bs[1], bs[2][0]
    return R, X, b


def _fold_weights(inp):
    """Build lhsT [72,128] and bias [128] for the fused per-tick matmul.

    rhs rows: [u_b0(16), u_b1(16), v_b0(16), v_b1(16), x_b0(4), x_b1(4)]
    out rows: kind-major: [ff1 b0,b1 | ff2 b0,b1 | z b0,b1 | z-dup b0,b1]
    state contribution: R @ h' = R @ 0.5*(v-u) -> +-0.5*R on v/u columns.
    """
    kinds = []
    for nm, scale in (("ff1", 1.0), ("ff2", 1.0), ("ta", 0.5)):
        Ws, bs = [], []
        for l in range(3):
            if nm == "ta":  # time gate: dense, wa+wb folded, x0.5 for tanh form
                W = 0.5 * (np.asarray(inp[f"ta_w{l}"], np.float64)
                           + np.asarray(inp[f"tb_w{l}"], np.float64))
                bb = 0.5 * (np.asarray(inp[f"ta_b{l}"], np.float64)
                            + np.asarray(inp[f"tb_b{l}"], np.float64))
            else:
                W = np.asarray(inp[f"{nm}_w{l}"], np.float64) * np.asarray(
                    inp[f"mask{l}"], np.float64)
                bb = np.asarray(inp[f"{nm}_b{l}"], np.float64)
            Ws.append(W)
            bs.append(bb)
        kinds.append(_make_rxb(Ws, bs))
    kinds.append(kinds[2])  # z duplicated

    W_big = np.zeros((128, 72), np.float64)
    bias = np.zeros((128,), np.float64)
    for ki, (R, X, b) in enumerate(kinds):
        for blk in range(G):
            m0 = ki * 32 + blk * 16
            W_big[m0:m0 + 16, blk * 16:blk * 16 + 16] = -0.5 * R
            W_big[m0:m0 + 16, 32 + blk * 16:32 + blk * 16 + 16] = 0.5 * R
            W_big[m0:m0 + 16, 64 + blk * 4:64 + blk * 4 + 4] = X
            bias[m0:m0 + 16] = b
    return np.ascontiguousarray(W_big.T, np.float32), bias.astype(np.float32)


def _prep_x(x):
    """x [B,T,4] -> per-core x_prep [S*NG, 8, 8*F]: group g=strm*NG+m holds
    rows (blk,feat) x (8 ticks, 128 samples), matching the ring x-row DMA."""
    xp = np.zeros((NCORES, BCORE, TICKS, IN), np.float32)
    xp[:, :, :T, :] = np.asarray(x, np.float32).reshape(NCORES, BCORE, T, IN)
    # (core, strm, blk, f, t, feat)
    xp = xp.reshape(NCORES, S, G, F, TICKS, IN)
    # -> (core, strm, t, blk, feat, f)
    xp = xp.transpose(0, 1, 4, 2, 5, 3)
    # -> (core, strm, m, t8, row=blk*4+feat, f)
    xp = xp.reshape(NCORES, S, NG, 8, G * IN, F)
    # -> (core, strm, m, row, t8, f)
    xp = xp.transpose(0, 1, 2, 4, 3, 5)
    return np.ascontiguousarray(xp.reshape(NCORES, S * NG, G * IN, 8 * F))


# ----------------------------------------------------------------------------
# device program
# ----------------------------------------------------------------------------

def _build():
    nc = bacc.Bacc("TRN2", target_bir_lowering=False, debug=False,
                   enable_asserts=True)
    x_d = nc.dram_tensor("x_prep", [S * NG, G * IN, 8 * F], FR,
                         kind="ExternalInput")
    w_d = nc.dram_tensor("lhsT", [72, 128], FR, kind="ExternalInput")
    b_d = nc.dram_tensor("bias", [128, 1], FP, kind="ExternalInput")
    sc_d = nc.dram_tensor("sscale", [128, 1], FP, kind="ExternalInput")
    sig_d = nc.dram_tensor("sig", [S, 128, NCHUNK * G * F], FP,
                           kind="ExternalOutput")
    hx_d = nc.dram_tensor("hx_raw", [S * 3, 64, F], FP, kind="ExternalOutput")
    x_ap, sig_ap, hx_ap = x_d.ap(), sig_d.ap(), hx_d.ap()

    with tile.TileContext(nc) as tc:
        with (
            tc.tile_pool(name="const", bufs=1) as cp,
            tc.tile_pool(name="ring", bufs=1) as rp,
            tc.tile_pool(name="stage", bufs=1) as sp,
            tc.tile_pool(name="y", bufs=6) as yp,
            tc.tile_pool(name="fin", bufs=1) as fp_,
            tc.tile_pool(name="ps", bufs=6, space=bass.MemorySpace.PSUM) as pp,
        ):
            lhsT = cp.tile([72, 128], FR, tag="lhsT")
            bias = cp.tile([128, 1], FP, tag="bias")
            sscale = cp.tile([128, 1], FP, tag="sscale")
            svec = cp.tile([64, 1], FP, tag="svec")
            nc.sync.dma_start(lhsT[:], w_d[:])
            nc.sync.dma_start(bias[:], b_d[:])
            nc.sync.dma_start(sscale[:], sc_d[:])
            nc.gpsimd.memset(svec[0:32, :], -1.0)
            nc.gpsimd.memset(svec[32:64, :], 1.0)

            rings = [[rp.tile([72, 8 * F], FR, name=f"ring{s}_{r}", tag=f"ring{s}_{r}")
                      for r in range(NRING)] for s in range(S)]
            # stage: per stream, u and v rows of the motor neuron per slot.
            # partition = slot % 128, free = chunk*256 + blk*128 + f
            st_u = [sp.tile([128, NCHUNK * G * F], FP, name=f"su{s}", tag=f"su{s}")
                    for s in range(S)]
            st_v = [sp.tile([128, NCHUNK * G * F], FP, name=f"sv{s}", tag=f"sv{s}")
                    for s in range(S)]
            for s in range(S):
                nc.gpsimd.memset(st_u[s][:], 0.0)
                nc.gpsimd.memset(st_v[s][:], 0.0)
                # uv(-1) = 0 lives in slot 0 of ring 0; zeros sourced from
                # the all-zero padded tail group of x_prep (8*1024 = 64*128)
                nc.sync.dma_start(rings[s][0][0:64, 0:F],
                                  x_ap[s * NG + NG - 1])
                # prefetch x for the first NRING ring periods
                for m in range(NRING):
                    nc.sync.dma_start(rings[s][m][64:72, :],
                                      x_ap[s * NG + m])

            def ring_slot(s, slot):
                rt = rings[s][(slot // 8) % NRING]
                off = (slot % 8) * F
                return rt, off

            for t in range(TICKS):
                for s in range(S):
                    rt, off = ring_slot(s, t)
                    ps = pp.tile([128, F], FP, tag="ps")
                    # float32r: single-pass PE matmul (fp32 lowers to two
                    # LDWEIGHTS+MATMUL passes, ~2x PE time on the cycle)
                    nc.tensor.matmul(ps[:], lhsT[:], rt[:, off:off + F],
                                     start=True, stop=True)
                    y = yp.tile([128, F], FP, tag="y")
                    nc.scalar.activation(y[:], ps[:],
                                         mybir.ActivationFunctionType.Tanh,
                                         bias=bias[:, 0:1])
                    if t < TICKS - 1:
                        ro, ooff = ring_slot(s, t + 1)
                        # realign z-tanh rows to base partition 0 (2-input
                        # SBUF ops require equal input base partitions)
                        yz = yp.tile([64, F], FP, tag="yz")
                        nc.vector.tensor_copy(yz[:], y[64:128, :])
                        nc.vector.scalar_tensor_tensor(
                            ro[0:64, ooff:ooff + F], yz[:],
                            svec[:, 0:1], y[0:64, :],
                            op0=mybir.AluOpType.add,
                            op1=mybir.AluOpType.mult)
                        if t in (0, 1):
                            # junk-state zeroing: h1',h2' of state(0) and
                            # h2' of state(1) must be 0. Source zeros from
                            # the all-zero padded tail group of x_prep.
                            rows = ((9, 16), (25, 32), (41, 48), (57, 64)) \
                                if t == 0 else ((15, 16), (31, 32),
                                                (47, 48), (63, 64))
                            for r0, r1 in rows:
                                nc.sync.dma_start(
                                    ro[r0:r1, ooff:ooff + F],
                                    x_ap[s * NG + NG - 1][0:r1 - r0, 0:F])
                    # end-of-period bookkeeping
                    if t % 8 == 7:
                        m = t // 8
                        # prefetch x for period m+NRING into this ring tile
                        if m + NRING < NG:
                            nc.sync.dma_start(rt[64:72, :],
                                              x_ap[s * NG + m + NRING])
                        # extract motor-neuron u,v rows for slots 8m..8m+7
                        chunk, p0 = (8 * m) // 128, (8 * m) % 128
                        for row, dst, doff in ((15, st_u[s], 0),
                                               (31, st_u[s], F),
                                               (47, st_v[s], 0),
                                               (63, st_v[s], F)):
                            base = chunk * G * F + doff
                            nc.sync.dma_start(
                                dst[p0:p0 + 8, base:base + F],
                                rt[row:row + 1, :].bitcast(FP))

            # final hidden states: uv(511)/uv(512)/uv(513) = slots 512..514
            for s in range(S):
                for k, slot in enumerate((512, 513, 514)):
                    rt, off = ring_slot(s, slot)
                    nc.sync.dma_start(hx_ap[s * 3 + k],
                                      rt[0:64, off:off + F].bitcast(FP))

            # final sigmoid pass: sigma = sigmoid(0.5*(v-u))*scale + MIN
            for s in range(S):
                o = fp_.tile([128, NCHUNK * G * F], FP, tag=f"o{s}")
                nc.vector.tensor_sub(o[:], st_v[s][:], st_u[s][:])
                g = fp_.tile([128, NCHUNK * G * F], FP, tag=f"g{s}")
                nc.scalar.activation(g[:], o[:],
                                     mybir.ActivationFunctionType.Sigmoid,
                                     scale=0.5)
                nc.vector.tensor_scalar(g[:], g[:], sscale[:, 0:1],
                                        SIGMA_MIN,
                                        op0=mybir.AluOpType.mult,
                                        op1=mybir.AluOpType.add)
                nc.sync.dma_start(sig_ap[s], g[:])

    nc.compile()
    return nc


def _get_compiled():
    global _compiled
    if _compiled is None:
        _compiled = _build()
    return _compiled


# ----------------------------------------------------------------------------
# host wrapper
# ----------------------------------------------------------------------------

def make_in_maps(inputs):
    lhsT, bias = _fold_weights(inputs)
    xp = _prep_x(inputs["x"])
    sscale = np.full((128, 1), np.float32(np.asarray(inputs["sigma_scale"])[0]),
                     np.float32)
    bias = bias.reshape(128, 1)
    return [
        {"x_prep": xp[c], "lhsT": lhsT, "bias": bias, "sscale": sscale}
        for c in range(NCORES)
    ]


def assemble_outputs(results):
    """results: list of 8 per-core dicts with 'sig' and 'hx_raw'."""
    sig_full = np.empty((B, T, 1), np.float32)
    hx_full = np.empty((B, 16), np.float32)
    for c, res in enumerate(results):
        sig = np.asarray(res["sig"], np.float32)  # [S,128,NCHUNK*2F]
        sig = sig.reshape(S, 128, NCHUNK, G, F)
        # (strm, blk, f, chunk, p) -> [b_local, slot]
        bs = sig.transpose(0, 3, 4, 2, 1).reshape(BCORE, NCHUNK * 128)
        sig_full[c * BCORE:(c + 1) * BCORE, :, 0] = bs[:, 3:3 + T]

        hx = np.asarray(res["hx_raw"], np.float32).reshape(S, 3, 64, F)
        h16 = 0.5 * (hx[:, :, 32:64, :] - hx[:, :, 0:32, :])  # [S,3,32,F]
        h16 = h16.reshape(S, 3, G, 16, F)
        for s in range(S):
            for blk in range(G):
                b0 = c * BCORE + s * G * F + blk * F
                hx_full[b0:b0 + F, 0:9] = h16[s, 0, blk, 0:9, :].T
                hx_full[b0:b0 + F, 9:15] = h16[s, 1, blk, 9:15, :].T
                hx_full[b0:b0 + F, 15:16] = h16[s, 2, blk, 15:16, :].T
    return sig_full, hx_full


def kernel(**inputs):
    nc = _get_compiled()
    in_maps = make_in_maps(inputs)
    res = bass_utils.run_bass_kernel_spmd(nc, in_maps,
                                          core_ids=list(range(NCORES)))
    return assemble_outputs(res.results)


# revision 13
# speedup vs baseline: 1.0037x; 1.0037x over previous
"""Trainium2 Bass kernel for nn_CfCDoseController.

Model: 3-layer CfC (closed-form continuous-time) RNN, AutoNCP(16,1) wiring:
  layers (fan_in, hidden): (4,9) -> (9,6) -> (6,1), T=512 steps, B=4096.
  cell: a = tanh(W1m@xx+b1); b = tanh(W2m@xx+b2); t = sigmoid(Wt@xx+bt)
        h' = a + t*(b-a)
  sigma = sigmoid(out)*sigma_scale + SIGMA_MIN;  hx = concat(final states)

Kernel strategy (pure data parallel over batch, 8 cores x 512 samples):
  * Layer-pipelined recurrence: tick s computes L0@time s, L1@s-1, L2@s-2.
    All three layers' cells then depend only on the previous tick's outputs,
    so they fuse into ONE matmul per tick.
  * sigmoid(z) = 0.5*(1 + tanh(z/2)) -> the time-gate shares the tanh op.
  * With y = tanh(z/2):  h' = 0.5*(v - u) where u = (y-1)*a, v = (y+1)*b.
    u,v are produced by a single fused DVE scalar_tensor_tensor op, and the
    0.5*(v-u) linear map is folded into the next matmul's weights.
  * Per core: 2 independent batch streams (latency hiding) x 2 blocks of 128
    samples packed on partitions. PSUM rows = [ff1(32), ff2(32), z(32), z(32)]
    (z duplicated so one STT covers both u and v), free dim = 128 samples.
  * Per stream-tick: 1 matmul + 1 tanh (ACT) + 1 STT (DVE). Everything else
    (x loads, output extraction, final sigmoid) is off the critical path.
"""

import numpy as np

try:
    import concourse.bass as bass
except Exception:  # pragma: no cover - path fallback
    import sys

    for _p in ("/opt/trn_rl_repo", "/root/.axon_site/_ro/trn_rl_repo"):
        if _p not in sys.path:
            sys.path.insert(0, _p)
    import concourse.bass as bass

import concourse.tile as tile
from concourse import bacc, bass_utils, mybir

SIGMA_MAX = 0.15
SIGMA_MIN = 0.001
LAYERS = [(4, 9), (9, 6), (6, 1)]
B, T, IN = 4096, 512, 4
NCORES = 8
BCORE = B // NCORES  # 512
S = 2  # independent streams per core
G = 2  # batch blocks per stream
F = BCORE // (S * G)  # 128 free dim (samples per block)
TICKS = 520  # 65 groups of 8 >= T + 2 pipeline tail
NG = TICKS // 8  # x/output DMA groups per stream
NRING = 6  # ring buffers per stream (8 slots each)
NCHUNK = 5  # ceil((TICKS+1)/128) slot chunks in the output stage
FP = mybir.dt.float32
FR = mybir.dt.float32r  # fp32 bits, single-pass PE matmul
BF = mybir.dt.bfloat16

_compiled = None


# ----------------------------------------------------------------------------
# host-side weight folding
# ----------------------------------------------------------------------------

def _make_rxb(Ws, bs):
    """Fold the 3 layers' (already masked) weights into the combined 16-dim
    state-space maps: pre = R @ state + X @ x + b, state=[h0(9),h1(6),h2(1)]."""
    R = np.zeros((16, 16), np.float64)
    X = np.zeros((16, 4), np.float64)
    b = np.zeros((16,), np.float64)
    W0, W1, W2 = Ws
    X[0:9, :] = W0[:, 0:4]
    R[0:9, 0:9] = W0[:, 4:13]
    R[9:15, 0:9] = W1[:, 0:9]
    R[9:15, 9:15] = W1[:, 9:15]
    R[15, 9:15] = W2[0, 0:6]
    R[15, 15] = W2[0, 6]
    b[0:9], b[9:15], b[15] = bs[0], bs[1], bs[2][0]
    return R, X, b


def _fold_weights(inp):
    """Build lhsT [72,128] and bias [128] for the fused per-tick matmul.

    rhs rows: [u_b0(16), u_b1(16), v_b0(16), v_b1(16), x_b0(4), x_b1(4)]
    out rows: kind-major: [ff1 b0,b1 | ff2 b0,b1 | z b0,b1 | z-dup b0,b1]
    state contribution: R @ h' = R @ 0.5*(v-u) -> +-0.5*R on v/u columns.
    """
    kinds = []
    for nm, scale in (("ff1", 1.0), ("ff2", 1.0), ("ta", 0.5)):
        Ws, bs = [], []
        for l in range(3):
            if nm == "ta":  # time gate: dense, wa+wb folded, x0.5 for tanh form
                W = 0.5 * (np.asarray(inp[f"ta_w{l}"], np.float64)
                           + np.asarray(inp[f"tb_w{l}"], np.float64))
                bb = 0.5 * (np.asarray(inp[f"ta_b{l}"], np.float64)
                            + np.asarray(inp[f"tb_b{l}"], np.float64))
            else:
                W = np.asarray(inp[f"{nm}_w{l}"], np.float64) * np.asarray(
                    inp[f"mask{l}"], np.float64)
                bb = np.asarray(inp[f"{nm}_b{l}"], np.float64)
            Ws.append(W)
            bs.append(bb)
        kinds.append(_make_rxb(Ws, bs))
    kinds.append(kinds[2])  # z duplicated

    W_big = np.zeros((128, 72), np.float64)
    bias = np.zeros((128,), np.float64)
    for ki, (R, X, b) in enumerate(kinds):
        for blk in range(G):
            m0 = ki * 32 + blk * 16
            W_big[m0:m0 + 16, blk * 16:blk * 16 + 16] = -0.5 * R
            W_big[m0:m0 + 16, 32 + blk * 16:32 + blk * 16 + 16] = 0.5 * R
            W_big[m0:m0 + 16, 64 + blk * 4:64 + blk * 4 + 4] = X
            bias[m0:m0 + 16] = b
    return np.ascontiguousarray(W_big.T, np.float32), bias.astype(np.float32)


def _prep_x(x):
    """x [B,T,4] -> per-core x_prep [S*NG, 8, 8*F]: group g=strm*NG+m holds
    rows (blk,feat) x (8 ticks, 128 samples), matching the ring x-row DMA."""
    xp = np.zeros((NCORES, BCORE, TICKS, IN), np.float32)
    xp[:, :, :T, :] = np.asarray(x, np.float32).reshape(NCORES, BCORE, T, IN)
    # (core, strm, blk, f, t, feat)
    xp = xp.reshape(NCORES, S, G, F, TICKS, IN)
    # -> (core, strm, t, blk, feat, f)
    xp = xp.transpose(0, 1, 4, 2, 5, 3)
    # -> (core, strm, m, t8, row=blk*4+feat, f)
    xp = xp.reshape(NCORES, S, NG, 8, G * IN, F)
    # -> (core, strm, m, row, t8, f)
    xp = xp.transpose(0, 1, 2, 4, 3, 5)
    return np.ascontiguousarray(xp.reshape(NCORES, S * NG, G * IN, 8 * F))


# ----------------------------------------------------------------------------
# device program
# ----------------------------------------------------------------------------

def _build():
    nc = bacc.Bacc("TRN2", target_bir_lowering=False, debug=False,
                   enable_asserts=True)
    x_d = nc.dram_tensor("x_prep", [S * NG, G * IN, 8 * F], FR,
                         kind="ExternalInput")
    w_d = nc.dram_tensor("lhsT", [72, 128], FR, kind="ExternalInput")
    b_d = nc.dram_tensor("bias", [128, 1], FP, kind="ExternalInput")
    sc_d = nc.dram_tensor("sscale", [128, 1], FP, kind="ExternalInput")
    sig_d = nc.dram_tensor("sig", [S, 128, NCHUNK * G * F], FP,
                           kind="ExternalOutput")
    hx_d = nc.dram_tensor("hx_raw", [S * 3, 64, F], FP, kind="ExternalOutput")
    warm_d = nc.dram_tensor("warm", [1, 128], FP, kind="ExternalOutput")
    x_ap, sig_ap, hx_ap = x_d.ap(), sig_d.ap(), hx_d.ap()

    with tile.TileContext(nc) as tc:
        with (
            tc.tile_pool(name="const", bufs=1) as cp,
            tc.tile_pool(name="ring", bufs=1) as rp,
            tc.tile_pool(name="stage", bufs=1) as sp,
            tc.tile_pool(name="y", bufs=6) as yp,
            tc.tile_pool(name="fin", bufs=1) as fp_,
            tc.tile_pool(name="ps", bufs=4, space=bass.MemorySpace.PSUM) as pp,
            tc.tile_pool(name="wp", bufs=1, space=bass.MemorySpace.PSUM) as wp,
        ):
            lhsT = cp.tile([72, 128], FR, tag="lhsT")
            bias = cp.tile([128, 1], FP, tag="bias")
            sscale = cp.tile([128, 1], FP, tag="sscale")
            svec = cp.tile([64, 1], FP, tag="svec")
            nc.sync.dma_start(lhsT[:], w_d[:])
            nc.sync.dma_start(bias[:], b_d[:])
            nc.sync.dma_start(sscale[:], sc_d[:])
            nc.gpsimd.memset(svec[0:32, :], -1.0)
            nc.gpsimd.memset(svec[32:64, :], 1.0)

            wps = wp.tile([1, 128], FP, name="wps", tag="wps")
            rings = [[rp.tile([72, 8 * F], FR, name=f"ring{s}_{r}", tag=f"ring{s}_{r}")
                      for r in range(NRING)] for s in range(S)]
            # stage: per stream, u and v rows of the motor neuron per slot.
            # partition = slot % 128, free = chunk*256 + blk*128 + f
            st_u = [sp.tile([128, NCHUNK * G * F], FP, name=f"su{s}", tag=f"su{s}")
                    for s in range(S)]
            st_v = [sp.tile([128, NCHUNK * G * F], FP, name=f"sv{s}", tag=f"sv{s}")
                    for s in range(S)]
            for s in range(S):
                nc.gpsimd.memset(st_u[s][:], 0.0)
                nc.gpsimd.memset(st_v[s][:], 0.0)
                # uv(-1) = 0 lives in slot 0 of ring 0; zeros sourced from
                # the all-zero padded tail group of x_prep (8*1024 = 64*128)
                nc.sync.dma_start(rings[s][0][0:64, 0:F],
                                  x_ap[s * NG + NG - 1])
                # prefetch x for the first NRING ring periods
                for m in range(NRING):
                    nc.sync.dma_start(rings[s][m][64:72, :],
                                      x_ap[s * NG + m])

            def ring_slot(s, slot):
                rt = rings[s][(slot // 8) % NRING]
                off = (slot % 8) * F
                return rt, off

            for t in range(TICKS):
                for s in range(S):
                    rt, off = ring_slot(s, t)
                    ps = pp.tile([128, F], FP, tag="ps")
                    # float32r: single-pass PE matmul (fp32 lowers to two
                    # LDWEIGHTS+MATMUL passes, ~2x PE time on the cycle)
                    nc.tensor.matmul(ps[:], lhsT[:], rt[:, off:off + F],
                                     start=True, stop=True)
                    # HAM warmers: dummy accumulating matmuls over the const
                    # weight tile keep PE duty high so the clock gate stays
                    # at full rate; accumulated + DMA'd out so they stay live
                    for _ in range(2):
                        nc.tensor.matmul(wps[:], lhsT[0:1, 0:1],
                                         lhsT[0:1, :],
                                         start=(t == 0 and s == 0 and _ == 0),
                                         stop=False, skip_group_check=True)
                    y = yp.tile([128, F], FP, tag="y")
                    nc.scalar.activation(y[:], ps[:],
                                         mybir.ActivationFunctionType.Tanh,
                                         bias=bias[:, 0:1])
                    if t < TICKS - 1:
                        ro, ooff = ring_slot(s, t + 1)
                        # realign z-tanh rows to base partition 0 (2-input
                        # SBUF ops require equal input base partitions)
                        yz = yp.tile([64, F], FP, tag="yz")
                        nc.vector.tensor_copy(yz[:], y[64:128, :])
                        nc.vector.scalar_tensor_tensor(
                            ro[0:64, ooff:ooff + F], yz[:],
                            svec[:, 0:1], y[0:64, :],
                            op0=mybir.AluOpType.add,
                            op1=mybir.AluOpType.mult)
                        if t in (0, 1):
                            # junk-state zeroing: h1',h2' of state(0) and
                            # h2' of state(1) must be 0. Source zeros from
                            # the all-zero padded tail group of x_prep.
                            rows = ((9, 16), (25, 32), (41, 48), (57, 64)) \
                                if t == 0 else ((15, 16), (31, 32),
                                                (47, 48), (63, 64))
                            for r0, r1 in rows:
                                nc.sync.dma_start(
                                    ro[r0:r1, ooff:ooff + F],
                                    x_ap[s * NG + NG - 1][0:r1 - r0, 0:F])
                    # end-of-period bookkeeping
                    if t % 8 == 7:
                        m = t // 8
                        # prefetch x for period m+NRING into this ring tile
                        if m + NRING < NG:
                            nc.sync.dma_start(rt[64:72, :],
                                              x_ap[s * NG + m + NRING])
                        # extract motor-neuron u,v rows for slots 8m..8m+7
                        chunk, p0 = (8 * m) // 128, (8 * m) % 128
                        for row, dst, doff in ((15, st_u[s], 0),
                                               (31, st_u[s], F),
                                               (47, st_v[s], 0),
                                               (63, st_v[s], F)):
                            base = chunk * G * F + doff
                            nc.sync.dma_start(
                                dst[p0:p0 + 8, base:base + F],
                                rt[row:row + 1, :].bitcast(FP))

            nc.tensor.matmul(wps[:], lhsT[0:1, 0:1], lhsT[0:1, :],
                             start=False, stop=True, skip_group_check=True)
            wsb = cp.tile([1, 128], FP, tag="wsb")
            nc.vector.tensor_copy(wsb[:], wps[:])
            nc.sync.dma_start(warm_d[:], wsb[:])

            # final hidden states: uv(511)/uv(512)/uv(513) = slots 512..514
            for s in range(S):
                for k, slot in enumerate((512, 513, 514)):
                    rt, off = ring_slot(s, slot)
                    nc.sync.dma_start(hx_ap[s * 3 + k],
                                      rt[0:64, off:off + F].bitcast(FP))

            # final sigmoid pass: sigma = sigmoid(0.5*(v-u))*scale + MIN
            for s in range(S):
                o = fp_.tile([128, NCHUNK * G * F], FP, tag=f"o{s}")
                nc.vector.tensor_sub(o[:], st_v[s][:], st_u[s][:])
                g = fp_.tile([128, NCHUNK * G * F], FP, tag=f"g{s}")
                nc.scalar.activation(g[:], o[:],
                                     mybir.ActivationFunctionType.Sigmoid,
                                     scale=0.5)
                nc.vector.tensor_scalar(g[:], g[:], sscale[:, 0:1],
                                        SIGMA_MIN,
                                        op0=mybir.AluOpType.mult,
                                        op1=mybir.AluOpType.add)
                nc.sync.dma_start(sig_ap[s], g[:])

    nc.compile()
    return nc


def _get_compiled():
    global _compiled
    if _compiled is None:
        _compiled = _build()
    return _compiled


# ----------------------------------------------------------------------------
# host wrapper
# ----------------------------------------------------------------------------

def make_in_maps(inputs):
    lhsT, bias = _fold_weights(inputs)
    xp = _prep_x(inputs["x"])
    sscale = np.full((128, 1), np.float32(np.asarray(inputs["sigma_scale"])[0]),
                     np.float32)
    bias = bias.reshape(128, 1)
    return [
        {"x_prep": xp[c], "lhsT": lhsT, "bias": bias, "sscale": sscale}
        for c in range(NCORES)
    ]


def assemble_outputs(results):
    """results: list of 8 per-core dicts with 'sig' and 'hx_raw'."""
    sig_full = np.empty((B, T, 1), np.float32)
    hx_full = np.empty((B, 16), np.float32)
    for c, res in enumerate(results):
        sig = np.asarray(res["sig"], np.float32)  # [S,128,NCHUNK*2F]
        sig = sig.reshape(S, 128, NCHUNK, G, F)
        # (strm, blk, f, chunk, p) -> [b_local, slot]
        bs = sig.transpose(0, 3, 4, 2, 1).reshape(BCORE, NCHUNK * 128)
        sig_full[c * BCORE:(c + 1) * BCORE, :, 0] = bs[:, 3:3 + T]

        hx = np.asarray(res["hx_raw"], np.float32).reshape(S, 3, 64, F)
        h16 = 0.5 * (hx[:, :, 32:64, :] - hx[:, :, 0:32, :])  # [S,3,32,F]
        h16 = h16.reshape(S, 3, G, 16, F)
        for s in range(S):
            for blk in range(G):
                b0 = c * BCORE + s * G * F + blk * F
                hx_full[b0:b0 + F, 0:9] = h16[s, 0, blk, 0:9, :].T
                hx_full[b0:b0 + F, 9:15] = h16[s, 1, blk, 9:15, :].T
                hx_full[b0:b0 + F, 15:16] = h16[s, 2, blk, 15:16, :].T
    return sig_full, hx_full


def kernel(**inputs):
    nc = _get_compiled()
    in_maps = make_in_maps(inputs)
    res = bass_utils.run_bass_kernel_spmd(nc, in_maps,
                                          core_ids=list(range(NCORES)))
    return assemble_outputs(res.results)
